# revision 8
# baseline (speedup 1.0000x reference)
"""Trainium2 Bass kernel for quaternion-algebra multi-head attention.

Math: algebra_linear(x, W, b) == x_flat @ M + b_flat where M[(n,j),(o,k)] =
sum_i C[i,j,k] W[o,n,i].  So the whole module is standard MHA with dense
1024x1024 projection matrices expanded on the host from the small algebra
weights.  Sharding: 8 cores = 2 batches x 4 head-groups (4 heads each).

Per-core device work (core c = (b, hg)):
  qT/kT [256, 2048] = Mq_shard^T-contraction against x^T (d on partitions)
  v     [2048, 256] natural layout, augmented with a ones column per head
  S^T[sk, sq] = kT.T @ qT per head (scale folded into Mq on host)
  expS = exp(S^T)  (no max subtraction: scores are bounded ~|4|, mask all-ones)
  ctx^T[65, sq] = [v | 1].T @ expS   -> rows 0:64 context, row 64 = softmax denom
  ctx_norm = ctx * (1/denom broadcast via K=1 matmul)
  out^T[1024, 2048] partial = Mo_shard^T-contraction against ctx_norm
Host gathers: out[b] = sum_hg out_hg^T.T (+ bo).
"""

import numpy as np

B, S, E = 2, 2048, 1024
NB = 256          # algebra blocks
MD = 4            # quaternion dim
H = 16            # total heads
HD = 64           # head dim
H_PER = 4         # heads per core
D = 256           # head dims per core (H_PER * HD)
P = 128
NE = E // P       # 8 e-chunks
ND = D // P       # 2 d-chunks per core
SQ_T = 512
NSQ = S // SQ_T   # 4
SK_T = 128
NSK = S // SK_T   # 16
NDO = E // P      # 8 out-dim chunks
SCALE = 1.0 / np.sqrt(HD)

_QUAT_TABLE = [
    (0, 0, 0, 1.0), (0, 1, 1, 1.0), (0, 2, 2, 1.0), (0, 3, 3, 1.0),
    (1, 0, 1, 1.0), (2, 0, 2, 1.0), (3, 0, 3, 1.0),
    (1, 1, 0, -1.0), (2, 2, 0, -1.0), (3, 3, 0, -1.0),
    (1, 2, 3, 1.0), (2, 1, 3, -1.0),
    (2, 3, 1, 1.0), (3, 2, 1, -1.0),
    (3, 1, 2, 1.0), (1, 3, 2, -1.0),
]


def _quat_C():
    C = np.zeros((4, 4, 4), dtype=np.float32)
    for i, j, k, s in _QUAT_TABLE:
        C[i, j, k] = s
    return C


def _expand(W, C):
    # W [NB, NB, 4] -> dense [E, E]:  y_flat = x_flat @ M
    Wm = np.einsum('oni,ijk->onjk', W.astype(np.float32), C)
    return np.ascontiguousarray(Wm.transpose(1, 2, 0, 3).reshape(E, E))


def _build_graph(with_qk_bias, with_v_bias, with_mask):
    import concourse.bacc as bacc
    import concourse.tile as tile
    import concourse.mybir as mybir

    f32 = mybir.dt.float32
    f32r = mybir.dt.float32r
    Exp = mybir.ActivationFunctionType.Exp
    Identity = mybir.ActivationFunctionType.Identity

    def v(ap):
        # DVE/ACT view of a float32r tile (same bits; PE reads it natively)
        return ap.bitcast(f32)

    nc = bacc.Bacc("TRN2", target_bir_lowering=False, debug=False, num_devices=8)

    xt_d = nc.dram_tensor("xt", [E, S], f32r, kind="ExternalInput").ap()
    wq_d = nc.dram_tensor("wq", [E, D], f32r, kind="ExternalInput").ap()
    wk_d = nc.dram_tensor("wk", [E, D], f32r, kind="ExternalInput").ap()
    wv_d = nc.dram_tensor("wv", [E, D], f32r, kind="ExternalInput").ap()
    wo_d = nc.dram_tensor("wo", [D, E], f32r, kind="ExternalInput").ap()
    out_d = nc.dram_tensor("out", [E, S], f32, kind="ExternalOutput").ap()
    if with_qk_bias:
        bq_d = nc.dram_tensor("bq", [D], f32, kind="ExternalInput").ap()
        bk_d = nc.dram_tensor("bk", [D], f32, kind="ExternalInput").ap()
    if with_v_bias:
        bv_d = nc.dram_tensor("bv", [D], f32r, kind="ExternalInput").ap()
    if with_mask:
        maskT_d = nc.dram_tensor("maskT", [S, S], f32, kind="ExternalInput").ap()

    with tile.TileContext(nc) as tc:
        import contextlib
        with nc.allow_low_precision(reason="float32r rounding of matmul operands"), \
                contextlib.ExitStack() as ctx:
            sing = ctx.enter_context(tc.tile_pool(name="sing", bufs=1))
            psum = ctx.enter_context(tc.tile_pool(name="psum", bufs=1, space="PSUM"))
            work = ctx.enter_context(tc.tile_pool(name="work", bufs=1))

            # ---- persistent SBUF tiles ----
            xt_sb = [
                sing.tile([P, S], f32r, name=f"xt{e}", tag=f"xt{e}")
                for e in range(NE)
            ]
            wq_sb = sing.tile([P, NE, D], f32r, name="wq_sb", tag="wq_sb")
            wk_sb = sing.tile([P, NE, D], f32r, name="wk_sb", tag="wk_sb")
            wv_sb = sing.tile([P, NE, D], f32r, name="wv_sb", tag="wv_sb")
            wo_sb = sing.tile([P, ND, E], f32r, name="wo_sb", tag="wo_sb")
            qT_sb = sing.tile([P, ND, S], f32r, name="qT_sb", tag="qT_sb")
            kT_sb = sing.tile([P, ND, S], f32r, name="kT_sb", tag="kT_sb")
            v_aug = sing.tile([P, NSK, H_PER, HD + 1], f32r, name="v_aug", tag="v_aug")
            ones_sb = sing.tile([1, HD], f32r, name="ones_sb", tag="ones_sb")

            # ---- input DMAs ----
            nc.sync.dma_start(wq_sb, wq_d.rearrange("(ko p) d -> p ko d", p=P))
            nc.sync.dma_start(wk_sb, wk_d.rearrange("(ko p) d -> p ko d", p=P))
            nc.sync.dma_start(wv_sb, wv_d.rearrange("(ko p) d -> p ko d", p=P))
            nc.sync.dma_start(wo_sb, wo_d.rearrange("(dk p) o -> p dk o", p=P))
            for e in range(NE):
                nc.sync.dma_start(xt_sb[e], xt_d[e * P:(e + 1) * P, :])

            ones_init = nc.inline_tensor(
                np.ones((P, NSK, H_PER, HD + 1), np.float32), name="ones_init").ap()
            ones_row = nc.inline_tensor(
                np.ones((1, P), np.float32), name="ones_row").ap()
            nc.sync.dma_start(v_aug, ones_init.bitcast(f32r))
            nc.sync.dma_start(ones_sb, ones_row[:, :HD].bitcast(f32r))

            if with_qk_bias:
                bq_sb = sing.tile([P, ND], f32, name="bq_sb", tag="bq_sb")
                bk_sb = sing.tile([P, ND], f32, name="bk_sb", tag="bk_sb")
                nc.sync.dma_start(bq_sb, bq_d.rearrange("(dk p) -> p dk", p=P))
                nc.sync.dma_start(bk_sb, bk_d.rearrange("(dk p) -> p dk", p=P))
            if with_v_bias:
                bv_row = sing.tile([1, D], f32r, name="bv_row", tag="bv_row")
                ones_r = sing.tile([1, P], f32r, name="ones_r", tag="ones_r")
                nc.sync.dma_start(bv_row, bv_d[None, :])
                nc.sync.dma_start(ones_r, ones_row.bitcast(f32r))

            # ---- q/k projections: qT[d, s] accumulated over e-chunks ----
            for wsb, dst, bias_sb in (
                (wq_sb, qT_sb, "bq_sb"),
                (wk_sb, kT_sb, "bk_sb"),
            ):
                for dk in range(ND):
                    for si in range(NSQ):
                        pp = psum.tile([P, SQ_T], f32, name="pp", tag="mm", bufs=4)
                        for e in range(NE):
                            nc.tensor.matmul(
                                pp,
                                lhsT=wsb[:, e, dk * P:(dk + 1) * P],
                                rhs=xt_sb[e][:, si * SQ_T:(si + 1) * SQ_T],
                                start=(e == 0),
                                stop=(e == NE - 1),
                            )
                        dslice = dst[:, dk, si * SQ_T:(si + 1) * SQ_T]
                        if with_qk_bias:
                            bb = bq_sb if bias_sb == "bq_sb" else bk_sb
                            nc.scalar.activation(
                                dslice, pp, Identity, bias=bb[:, dk:dk + 1])
                        else:
                            nc.vector.tensor_copy(dslice, pp)

            # ---- v projection: natural [s, d] layout into v_aug ----
            for st in range(NSK):
                pv = psum.tile([P, D], f32, name="pv", tag="mm", bufs=4)
                n_acc = NE + (1 if with_v_bias else 0)
                for e in range(NE):
                    nc.tensor.matmul(
                        pv,
                        lhsT=xt_sb[e][:, st * P:(st + 1) * P],
                        rhs=wv_sb[:, e, :],
                        start=(e == 0),
                        stop=(e == n_acc - 1),
                    )
                if with_v_bias:
                    nc.tensor.matmul(pv, lhsT=ones_r, rhs=bv_row,
                                     start=False, stop=True)
                for h in range(H_PER):
                    nc.vector.tensor_copy(
                        v_aug[:, st, h, 0:HD], pv[:, h * HD:(h + 1) * HD])

            # ---- attention + out-projection, per sq block ----
            for si in range(NSQ):
                sq = slice(si * SQ_T, (si + 1) * SQ_T)
                ctxT = work.tile([P, ND, SQ_T], f32r, name="ctxT", tag="ctxT", bufs=2)
                for h in range(H_PER):
                    dk, po = h // 2, (h % 2) * HD
                    pc = psum.tile([HD + 1, SQ_T], f32, name="pc", tag="pc", bufs=2)
                    for sk in range(NSK):
                        ps = psum.tile([P, SQ_T], f32, name="ps", tag="mm", bufs=4)
                        nc.tensor.matmul(
                            ps,
                            lhsT=kT_sb[po:po + HD, dk, sk * SK_T:(sk + 1) * SK_T],
                            rhs=qT_sb[po:po + HD, dk, sq],
                            start=True, stop=True,
                        )
                        ex = work.tile([P, SQ_T], f32r, name="ex", tag="ex", bufs=6)
                        nc.scalar.activation(ex, ps, Exp)
                        if with_mask:
                            mt = work.tile([P, SQ_T], f32, name="mt", tag="mt", bufs=4)
                            nc.sync.dma_start(
                                mt, maskT_d[sk * SK_T:(sk + 1) * SK_T, sq])
                            nc.vector.tensor_mul(ex, v(ex), mt)
                        nc.tensor.matmul(
                            pc, lhsT=v_aug[:, sk, h, :], rhs=ex,
                            start=(sk == 0), stop=(sk == NSK - 1),
                        )
                    rec = work.tile([1, SQ_T], f32r, name="rec", tag="rec", bufs=2)
                    nc.vector.reciprocal(rec, pc[HD:HD + 1, :])
                    pb = psum.tile([HD, SQ_T], f32, name="pb", tag="pb", bufs=1)
                    nc.tensor.matmul(pb, lhsT=ones_sb, rhs=rec, start=True, stop=True)
                    cu = work.tile([HD, SQ_T], f32, name="cu", tag="cu", bufs=2)
                    nc.vector.tensor_copy(cu, pc[0:HD, :])
                    nc.vector.tensor_mul(ctxT[po:po + HD, dk, :], cu, pb)

                for do in range(NDO):
                    pu = psum.tile([P, SQ_T], f32, name="pu", tag="mm", bufs=4)
                    for dk in range(ND):
                        nc.tensor.matmul(
                            pu,
                            lhsT=wo_sb[:, dk, do * P:(do + 1) * P],
                            rhs=ctxT[:, dk, :],
                            start=(dk == 0), stop=(dk == ND - 1),
                        )
                    ot = work.tile([P, SQ_T], f32, name="ot", tag="ot", bufs=3)
                    nc.vector.tensor_copy(ot, pu)
                    nc.sync.dma_start(out_d[do * P:(do + 1) * P, sq], ot)

    nc.compile()
    return nc


_GRAPH_CACHE = {}


def kernel(x, mask, Wq, bq, Wk, bk, Wv, bv, Wo, bo):
    from concourse.bass_utils import run_bass_kernel_spmd

    x = np.asarray(x, dtype=np.float32)
    mask = np.asarray(mask)
    C = _quat_C()
    Mq = _expand(np.asarray(Wq), C) * SCALE
    Mk = _expand(np.asarray(Wk), C)
    Mv = _expand(np.asarray(Wv), C)
    Mo = _expand(np.asarray(Wo), C)
    bq_f = np.asarray(bq, np.float32).reshape(-1) * SCALE
    bk_f = np.asarray(bk, np.float32).reshape(-1)
    bv_f = np.asarray(bv, np.float32).reshape(-1)
    bo_f = np.asarray(bo, np.float32).reshape(-1)

    with_qk_bias = bool(np.any(bq_f) or np.any(bk_f))
    with_v_bias = bool(np.any(bv_f))
    with_mask = bool(np.any(np.asarray(mask) == 0))

    key = (with_qk_bias, with_v_bias, with_mask)
    if key not in _GRAPH_CACHE:
        _GRAPH_CACHE[key] = _build_graph(*key)
    nc = _GRAPH_CACHE[key]

    if with_mask:
        maskT = np.ascontiguousarray(
            np.broadcast_to(mask, (1, 1, S, S))[0, 0].T.astype(np.float32))

    in_maps = []
    for core in range(8):
        b, hg = core // 4, core % 4
        cs = slice(hg * D, (hg + 1) * D)
        m = {
            "xt": np.ascontiguousarray(x[b].T),
            "wq": np.ascontiguousarray(Mq[:, cs]),
            "wk": np.ascontiguousarray(Mk[:, cs]),
            "wv": np.ascontiguousarray(Mv[:, cs]),
            "wo": np.ascontiguousarray(Mo[cs, :]),
        }
        if with_qk_bias:
            m["bq"] = np.ascontiguousarray(bq_f[cs])
            m["bk"] = np.ascontiguousarray(bk_f[cs])
        if with_v_bias:
            m["bv"] = np.ascontiguousarray(bv_f[cs])
        if with_mask:
            m["maskT"] = maskT
        in_maps.append(m)

    res = run_bass_kernel_spmd(nc, in_maps, core_ids=list(range(8))).results

    out = np.zeros((B, S, E), dtype=np.float32)
    for core in range(8):
        b = core // 4
        out[b] += res[core]["out"].T
    out += bo_f
    return out


# revision 10
# speedup vs baseline: 1.1730x; 1.1730x over previous
"""Trainium2 Bass kernel for quaternion-algebra multi-head attention.

Math: algebra_linear(x, W, b) == x_flat @ M + b_flat where M[(n,j),(o,k)] =
sum_i C[i,j,k] W[o,n,i].  So the whole module is standard MHA with dense
1024x1024 projection matrices expanded on the host from the small algebra
weights.  Sharding: 8 cores = 2 batches x 4 head-groups (4 heads each).

Per-core device work (core c = (b, hg)):
  qT/kT [256, 2048] = Mq_shard^T-contraction against x^T (d on partitions)
  v     [2048, 256] natural layout, augmented with a ones column per head
  S^T[sk, sq] = kT.T @ qT per head (scale folded into Mq on host)
  expS = exp(S^T)  (no max subtraction: scores are bounded ~|4|, mask all-ones)
  ctx^T[65, sq] = [v | 1].T @ expS   -> rows 0:64 context, row 64 = softmax denom
  ctx_norm = ctx * (1/denom broadcast via K=1 matmul)
  out^T[1024, 2048] partial = Mo_shard^T-contraction against ctx_norm
Host gathers: out[b] = sum_hg out_hg^T.T (+ bo).
"""

import numpy as np
import ml_dtypes

B, S, E = 2, 2048, 1024
NB = 256          # algebra blocks
MD = 4            # quaternion dim
H = 16            # total heads
HD = 64           # head dim
H_PER = 4         # heads per core
D = 256           # head dims per core (H_PER * HD)
P = 128
NE = E // P       # 8 e-chunks
ND = D // P       # 2 d-chunks per core
SQ_T = 512
NSQ = S // SQ_T   # 4
SK_T = 128
NSK = S // SK_T   # 16
NDO = E // P      # 8 out-dim chunks
SCALE = 1.0 / np.sqrt(HD)

_QUAT_TABLE = [
    (0, 0, 0, 1.0), (0, 1, 1, 1.0), (0, 2, 2, 1.0), (0, 3, 3, 1.0),
    (1, 0, 1, 1.0), (2, 0, 2, 1.0), (3, 0, 3, 1.0),
    (1, 1, 0, -1.0), (2, 2, 0, -1.0), (3, 3, 0, -1.0),
    (1, 2, 3, 1.0), (2, 1, 3, -1.0),
    (2, 3, 1, 1.0), (3, 2, 1, -1.0),
    (3, 1, 2, 1.0), (1, 3, 2, -1.0),
]


def _quat_C():
    C = np.zeros((4, 4, 4), dtype=np.float32)
    for i, j, k, s in _QUAT_TABLE:
        C[i, j, k] = s
    return C


def _expand(W, C):
    # W [NB, NB, 4] -> dense [E, E]:  y_flat = x_flat @ M
    Wm = np.einsum('oni,ijk->onjk', W.astype(np.float32), C)
    return np.ascontiguousarray(Wm.transpose(1, 2, 0, 3).reshape(E, E))


def _build_graph(with_qk_bias, with_v_bias, with_mask):
    import concourse.bacc as bacc
    import concourse.tile as tile
    import concourse.mybir as mybir

    f32 = mybir.dt.float32
    bf16 = mybir.dt.bfloat16
    Exp = mybir.ActivationFunctionType.Exp
    Identity = mybir.ActivationFunctionType.Identity

    nc = bacc.Bacc("TRN2", target_bir_lowering=False, debug=False, num_devices=8)

    xt_d = nc.dram_tensor("xt", [E, S], bf16, kind="ExternalInput").ap()
    wq_d = nc.dram_tensor("wq", [E, D], bf16, kind="ExternalInput").ap()
    wk_d = nc.dram_tensor("wk", [E, D], bf16, kind="ExternalInput").ap()
    wv_d = nc.dram_tensor("wv", [E, D], bf16, kind="ExternalInput").ap()
    wo_d = nc.dram_tensor("wo", [D, E], bf16, kind="ExternalInput").ap()
    out_d = nc.dram_tensor("out", [E, S], f32, kind="ExternalOutput").ap()
    if with_qk_bias:
        bq_d = nc.dram_tensor("bq", [D], f32, kind="ExternalInput").ap()
        bk_d = nc.dram_tensor("bk", [D], f32, kind="ExternalInput").ap()
    if with_v_bias:
        bv_d = nc.dram_tensor("bv", [D], bf16, kind="ExternalInput").ap()
    if with_mask:
        maskT_d = nc.dram_tensor("maskT", [S, S], bf16, kind="ExternalInput").ap()

    with tile.TileContext(nc) as tc:
        import contextlib
        with nc.allow_low_precision(reason="float32r rounding of matmul operands"), \
                contextlib.ExitStack() as ctx:
            sing = ctx.enter_context(tc.tile_pool(name="sing", bufs=1))
            psum = ctx.enter_context(tc.tile_pool(name="psum", bufs=1, space="PSUM"))
            work = ctx.enter_context(tc.tile_pool(name="work", bufs=1))

            # ---- persistent SBUF tiles ----
            xt_sb = [
                sing.tile([P, S], bf16, name=f"xt{e}", tag=f"xt{e}")
                for e in range(NE)
            ]
            wq_sb = sing.tile([P, NE, D], bf16, name="wq_sb", tag="wq_sb")
            wk_sb = sing.tile([P, NE, D], bf16, name="wk_sb", tag="wk_sb")
            wv_sb = sing.tile([P, NE, D], bf16, name="wv_sb", tag="wv_sb")
            wo_sb = sing.tile([P, ND, E], bf16, name="wo_sb", tag="wo_sb")
            qT_sb = sing.tile([P, ND, S], bf16, name="qT_sb", tag="qT_sb")
            kT_sb = sing.tile([P, ND, S], bf16, name="kT_sb", tag="kT_sb")
            v_aug = sing.tile([P, NSK, H_PER, HD + 1], bf16, name="v_aug", tag="v_aug")
            ones_sb = sing.tile([1, HD], bf16, name="ones_sb", tag="ones_sb")

            # ---- input DMAs ----
            nc.sync.dma_start(wq_sb, wq_d.rearrange("(ko p) d -> p ko d", p=P))
            nc.sync.dma_start(wk_sb, wk_d.rearrange("(ko p) d -> p ko d", p=P))
            nc.sync.dma_start(wv_sb, wv_d.rearrange("(ko p) d -> p ko d", p=P))
            nc.sync.dma_start(wo_sb, wo_d.rearrange("(dk p) o -> p dk o", p=P))
            for e in range(NE):
                nc.sync.dma_start(xt_sb[e], xt_d[e * P:(e + 1) * P, :])

            import ml_dtypes
            ones_init = nc.inline_tensor(
                np.ones((P, NSK, H_PER, HD + 1), ml_dtypes.bfloat16), name="ones_init").ap()
            ones_row = nc.inline_tensor(
                np.ones((1, P), ml_dtypes.bfloat16), name="ones_row").ap()
            nc.sync.dma_start(v_aug, ones_init)
            nc.sync.dma_start(ones_sb, ones_row[:, :HD])

            if with_qk_bias:
                bq_sb = sing.tile([P, ND], f32, name="bq_sb", tag="bq_sb")
                bk_sb = sing.tile([P, ND], f32, name="bk_sb", tag="bk_sb")
                nc.sync.dma_start(bq_sb, bq_d.rearrange("(dk p) -> p dk", p=P))
                nc.sync.dma_start(bk_sb, bk_d.rearrange("(dk p) -> p dk", p=P))
            if with_v_bias:
                bv_row = sing.tile([1, D], bf16, name="bv_row", tag="bv_row")
                ones_r = sing.tile([1, P], bf16, name="ones_r", tag="ones_r")
                nc.sync.dma_start(bv_row, bv_d[None, :])
                nc.sync.dma_start(ones_r, ones_row)

            # ---- q/k projections: qT[d, s] accumulated over e-chunks ----
            for wsb, dst, bias_sb in (
                (wq_sb, qT_sb, "bq_sb"),
                (wk_sb, kT_sb, "bk_sb"),
            ):
                for dk in range(ND):
                    for si in range(NSQ):
                        pp = psum.tile([P, SQ_T], f32, name="pp", tag="mm", bufs=4)
                        for e in range(NE):
                            nc.tensor.matmul(
                                pp,
                                lhsT=wsb[:, e, dk * P:(dk + 1) * P],
                                rhs=xt_sb[e][:, si * SQ_T:(si + 1) * SQ_T],
                                start=(e == 0),
                                stop=(e == NE - 1),
                            )
                        dslice = dst[:, dk, si * SQ_T:(si + 1) * SQ_T]
                        if with_qk_bias:
                            bb = bq_sb if bias_sb == "bq_sb" else bk_sb
                            nc.scalar.activation(
                                dslice, pp, Identity, bias=bb[:, dk:dk + 1])
                        else:
                            nc.vector.tensor_copy(dslice, pp)

            # ---- v projection: natural [s, d] layout into v_aug ----
            for st in range(NSK):
                pv = psum.tile([P, D], f32, name="pv", tag="mm", bufs=4)
                n_acc = NE + (1 if with_v_bias else 0)
                for e in range(NE):
                    nc.tensor.matmul(
                        pv,
                        lhsT=xt_sb[e][:, st * P:(st + 1) * P],
                        rhs=wv_sb[:, e, :],
                        start=(e == 0),
                        stop=(e == n_acc - 1),
                    )
                if with_v_bias:
                    nc.tensor.matmul(pv, lhsT=ones_r, rhs=bv_row,
                                     start=False, stop=True)
                for h in range(H_PER):
                    nc.vector.tensor_copy(
                        v_aug[:, st, h, 0:HD], pv[:, h * HD:(h + 1) * HD])

            # ---- attention + out-projection, per sq block ----
            for si in range(NSQ):
                sq = slice(si * SQ_T, (si + 1) * SQ_T)
                ctxT = work.tile([P, ND, SQ_T], bf16, name="ctxT", tag="ctxT", bufs=2)
                for h in range(H_PER):
                    dk, po = h // 2, (h % 2) * HD
                    pc = psum.tile([HD + 1, SQ_T], f32, name="pc", tag="pc", bufs=2)
                    for sk in range(NSK):
                        ps = psum.tile([P, SQ_T], f32, name="ps", tag="mm", bufs=4)
                        nc.tensor.matmul(
                            ps,
                            lhsT=kT_sb[po:po + HD, dk, sk * SK_T:(sk + 1) * SK_T],
                            rhs=qT_sb[po:po + HD, dk, sq],
                            start=True, stop=True,
                        )
                        ex = work.tile([P, SQ_T], bf16, name="ex", tag="ex", bufs=6)
                        nc.scalar.activation(ex, ps, Exp)
                        if with_mask:
                            mt = work.tile([P, SQ_T], bf16, name="mt", tag="mt", bufs=4)
                            nc.sync.dma_start(
                                mt, maskT_d[sk * SK_T:(sk + 1) * SK_T, sq])
                            nc.vector.tensor_mul(ex, ex, mt)
                        nc.tensor.matmul(
                            pc, lhsT=v_aug[:, sk, h, :], rhs=ex,
                            start=(sk == 0), stop=(sk == NSK - 1),
                        )
                    rec = work.tile([1, SQ_T], bf16, name="rec", tag="rec", bufs=2)
                    nc.vector.reciprocal(rec, pc[HD:HD + 1, :])
                    pb = psum.tile([HD, SQ_T], f32, name="pb", tag="pb", bufs=1)
                    nc.tensor.matmul(pb, lhsT=ones_sb, rhs=rec, start=True, stop=True)
                    cu = work.tile([HD, SQ_T], f32, name="cu", tag="cu", bufs=2)
                    nc.vector.tensor_copy(cu, pc[0:HD, :])
                    nc.vector.tensor_mul(ctxT[po:po + HD, dk, :], cu, pb)

                for do in range(NDO):
                    pu = psum.tile([P, SQ_T], f32, name="pu", tag="mm", bufs=4)
                    for dk in range(ND):
                        nc.tensor.matmul(
                            pu,
                            lhsT=wo_sb[:, dk, do * P:(do + 1) * P],
                            rhs=ctxT[:, dk, :],
                            start=(dk == 0), stop=(dk == ND - 1),
                        )
                    ot = work.tile([P, SQ_T], f32, name="ot", tag="ot", bufs=3)
                    nc.vector.tensor_copy(ot, pu)
                    nc.sync.dma_start(out_d[do * P:(do + 1) * P, sq], ot)

    nc.compile()
    return nc


_GRAPH_CACHE = {}


def kernel(x, mask, Wq, bq, Wk, bk, Wv, bv, Wo, bo):
    from concourse.bass_utils import run_bass_kernel_spmd

    x = np.asarray(x, dtype=np.float32)
    mask = np.asarray(mask)
    C = _quat_C()
    Mq = _expand(np.asarray(Wq), C) * SCALE
    Mk = _expand(np.asarray(Wk), C)
    Mv = _expand(np.asarray(Wv), C)
    Mo = _expand(np.asarray(Wo), C)
    bq_f = np.asarray(bq, np.float32).reshape(-1) * SCALE
    bk_f = np.asarray(bk, np.float32).reshape(-1)
    bv_f = np.asarray(bv, np.float32).reshape(-1)
    bo_f = np.asarray(bo, np.float32).reshape(-1)

    with_qk_bias = bool(np.any(bq_f) or np.any(bk_f))
    with_v_bias = bool(np.any(bv_f))
    with_mask = bool(np.any(np.asarray(mask) == 0))

    key = (with_qk_bias, with_v_bias, with_mask)
    if key not in _GRAPH_CACHE:
        _GRAPH_CACHE[key] = _build_graph(*key)
    nc = _GRAPH_CACHE[key]

    if with_mask:
        maskT = np.ascontiguousarray(
            np.broadcast_to(mask, (1, 1, S, S))[0, 0].T.astype(ml_dtypes.bfloat16))

    in_maps = []
    for core in range(8):
        b, hg = core // 4, core % 4
        cs = slice(hg * D, (hg + 1) * D)
        m = {
            "xt": np.ascontiguousarray(x[b].T.astype(ml_dtypes.bfloat16)),
            "wq": np.ascontiguousarray(Mq[:, cs].astype(ml_dtypes.bfloat16)),
            "wk": np.ascontiguousarray(Mk[:, cs].astype(ml_dtypes.bfloat16)),
            "wv": np.ascontiguousarray(Mv[:, cs].astype(ml_dtypes.bfloat16)),
            "wo": np.ascontiguousarray(Mo[cs, :].astype(ml_dtypes.bfloat16)),
        }
        if with_qk_bias:
            m["bq"] = np.ascontiguousarray(bq_f[cs])
            m["bk"] = np.ascontiguousarray(bk_f[cs])
        if with_v_bias:
            m["bv"] = np.ascontiguousarray(bv_f[cs].astype(ml_dtypes.bfloat16))
        if with_mask:
            m["maskT"] = maskT
        in_maps.append(m)

    res = run_bass_kernel_spmd(nc, in_maps, core_ids=list(range(8))).results

    out = np.zeros((B, S, E), dtype=np.float32)
    for core in range(8):
        b = core // 4
        out[b] += res[core]["out"].T
    out += bo_f
    return out


# revision 12
# speedup vs baseline: 1.3916x; 1.1863x over previous
"""Trainium2 Bass kernel for quaternion-algebra multi-head attention.

Math: algebra_linear(x, W, b) == x_flat @ M + b_flat where M[(n,j),(o,k)] =
sum_i C[i,j,k] W[o,n,i].  So the whole module is standard MHA with dense
1024x1024 projection matrices expanded on the host from the small algebra
weights.  Sharding: 8 cores = 2 batches x 4 head-groups (4 heads each).

Per-core device work (core c = (b, hg)):
  qT/kT [256, 2048] = Mq_shard^T-contraction against x^T (d on partitions)
  v     [2048, 256] natural layout, augmented with a ones column per head
  S^T[sk, sq] = kT.T @ qT per head (scale folded into Mq on host)
  expS = exp(S^T)  (no max subtraction: scores are bounded ~|4|, mask all-ones)
  ctx^T[65, sq] = [v | 1].T @ expS   -> rows 0:64 context, row 64 = softmax denom
  ctx_norm = ctx * (1/denom broadcast via K=1 matmul)
  out^T[1024, 2048] partial = Mo_shard^T-contraction against ctx_norm
Host gathers: out[b] = sum_hg out_hg^T.T (+ bo).
"""

import numpy as np
import ml_dtypes

B, S, E = 2, 2048, 1024
NB = 256          # algebra blocks
MD = 4            # quaternion dim
H = 16            # total heads
HD = 64           # head dim
H_PER = 4         # heads per core
D = 256           # head dims per core (H_PER * HD)
P = 128
NE = E // P       # 8 e-chunks
ND = D // P       # 2 d-chunks per core
SQ_T = 512
NSQ = S // SQ_T   # 4
SK_T = 128
NSK = S // SK_T   # 16
NDO = E // P      # 8 out-dim chunks
SCALE = 1.0 / np.sqrt(HD)

_QUAT_TABLE = [
    (0, 0, 0, 1.0), (0, 1, 1, 1.0), (0, 2, 2, 1.0), (0, 3, 3, 1.0),
    (1, 0, 1, 1.0), (2, 0, 2, 1.0), (3, 0, 3, 1.0),
    (1, 1, 0, -1.0), (2, 2, 0, -1.0), (3, 3, 0, -1.0),
    (1, 2, 3, 1.0), (2, 1, 3, -1.0),
    (2, 3, 1, 1.0), (3, 2, 1, -1.0),
    (3, 1, 2, 1.0), (1, 3, 2, -1.0),
]


def _quat_C():
    C = np.zeros((4, 4, 4), dtype=np.float32)
    for i, j, k, s in _QUAT_TABLE:
        C[i, j, k] = s
    return C


def _expand(W, C):
    # W [NB, NB, 4] -> dense [E, E]:  y_flat = x_flat @ M
    Wm = np.einsum('oni,ijk->onjk', W.astype(np.float32), C)
    return np.ascontiguousarray(Wm.transpose(1, 2, 0, 3).reshape(E, E))


def _build_graph(with_qk_bias, with_v_bias, with_mask):
    import concourse.bacc as bacc
    import concourse.tile as tile
    import concourse.mybir as mybir

    f32 = mybir.dt.float32
    bf16 = mybir.dt.bfloat16
    Exp = mybir.ActivationFunctionType.Exp
    Identity = mybir.ActivationFunctionType.Identity

    nc = bacc.Bacc("TRN2", target_bir_lowering=False, debug=False, num_devices=8)

    xt_d = nc.dram_tensor("xt", [E, S], bf16, kind="ExternalInput").ap()
    wq_d = nc.dram_tensor("wq", [E, D], bf16, kind="ExternalInput").ap()
    wk_d = nc.dram_tensor("wk", [E, D], bf16, kind="ExternalInput").ap()
    wv_d = nc.dram_tensor("wv", [E, D], bf16, kind="ExternalInput").ap()
    wo_d = nc.dram_tensor("wo", [D, E], bf16, kind="ExternalInput").ap()
    out_d = nc.dram_tensor("out", [E, S], f32, kind="ExternalOutput").ap()
    if with_qk_bias:
        bq_d = nc.dram_tensor("bq", [D], f32, kind="ExternalInput").ap()
        bk_d = nc.dram_tensor("bk", [D], f32, kind="ExternalInput").ap()
    if with_v_bias:
        bv_d = nc.dram_tensor("bv", [D], bf16, kind="ExternalInput").ap()
    if with_mask:
        maskT_d = nc.dram_tensor("maskT", [S, S], bf16, kind="ExternalInput").ap()

    with tile.TileContext(nc) as tc:
        import contextlib
        with nc.allow_low_precision(reason="float32r rounding of matmul operands"), \
                contextlib.ExitStack() as ctx:
            sing = ctx.enter_context(tc.tile_pool(name="sing", bufs=1))
            psum = ctx.enter_context(tc.tile_pool(name="psum", bufs=1, space="PSUM"))
            work = ctx.enter_context(tc.tile_pool(name="work", bufs=1))

            # ---- persistent SBUF tiles ----
            xt_sb = [
                sing.tile([P, S], bf16, name=f"xt{e}", tag=f"xt{e}")
                for e in range(NE)
            ]
            wq_sb = sing.tile([P, NE, D], bf16, name="wq_sb", tag="wq_sb")
            wk_sb = sing.tile([P, NE, D], bf16, name="wk_sb", tag="wk_sb")
            wv_sb = sing.tile([P, NE, D], bf16, name="wv_sb", tag="wv_sb")
            wo_sb = sing.tile([P, ND, E], bf16, name="wo_sb", tag="wo_sb")
            qT_sb = sing.tile([P, ND, S], bf16, name="qT_sb", tag="qT_sb")
            kT_sb = sing.tile([P, H_PER, S], bf16, name="kT_sb", tag="kT_sb")
            nc.vector.memset(kT_sb, 0.0)
            v_aug = sing.tile([P, NSK, H_PER, HD + 1], bf16, name="v_aug", tag="v_aug")
            ones_sb = sing.tile([1, HD], bf16, name="ones_sb", tag="ones_sb")

            # ---- input DMAs ----
            nc.sync.dma_start(wq_sb, wq_d.rearrange("(ko p) d -> p ko d", p=P))
            nc.sync.dma_start(wk_sb, wk_d.rearrange("(ko p) d -> p ko d", p=P))
            nc.sync.dma_start(wv_sb, wv_d.rearrange("(ko p) d -> p ko d", p=P))
            nc.sync.dma_start(wo_sb, wo_d.rearrange("(dk p) o -> p dk o", p=P))
            for e in range(NE):
                nc.sync.dma_start(xt_sb[e], xt_d[e * P:(e + 1) * P, :])

            import ml_dtypes
            ones_init = nc.inline_tensor(
                np.ones((P, NSK, H_PER, HD + 1), ml_dtypes.bfloat16), name="ones_init").ap()
            ones_row = nc.inline_tensor(
                np.ones((1, P), ml_dtypes.bfloat16), name="ones_row").ap()
            nc.sync.dma_start(v_aug, ones_init)
            nc.sync.dma_start(ones_sb, ones_row[:, :HD])

            if with_qk_bias:
                bq_sb = sing.tile([P, ND], f32, name="bq_sb", tag="bq_sb")
                bk_sb = sing.tile([P, ND], f32, name="bk_sb", tag="bk_sb")
                nc.sync.dma_start(bq_sb, bq_d.rearrange("(dk p) -> p dk", p=P))
                nc.sync.dma_start(bk_sb, bk_d.rearrange("(dk p) -> p dk", p=P))
            if with_v_bias:
                bv_row = sing.tile([1, D], bf16, name="bv_row", tag="bv_row")
                ones_r = sing.tile([1, P], bf16, name="ones_r", tag="ones_r")
                nc.sync.dma_start(bv_row, bv_d[None, :])
                nc.sync.dma_start(ones_r, ones_row)

            # ---- q/k projections: qT[d, s] accumulated over e-chunks ----
            for wsb, dst, bias_sb in (
                (wq_sb, qT_sb, "bq_sb"),
                (wk_sb, kT_sb, "bk_sb"),
            ):
                for dk in range(ND):
                    for si in range(NSQ):
                        pp = psum.tile([P, SQ_T], f32, name="pp", tag="mm", bufs=4)
                        for e in range(NE):
                            nc.tensor.matmul(
                                pp,
                                lhsT=wsb[:, e, dk * P:(dk + 1) * P],
                                rhs=xt_sb[e][:, si * SQ_T:(si + 1) * SQ_T],
                                start=(e == 0),
                                stop=(e == NE - 1),
                            )
                        if dst is qT_sb:
                            dslice = dst[:, dk, si * SQ_T:(si + 1) * SQ_T]
                            if with_qk_bias:
                                nc.scalar.activation(
                                    dslice, pp, Identity, bias=bq_sb[:, dk:dk + 1])
                            else:
                                nc.vector.tensor_copy(dslice, pp)
                        else:
                            # kT: split the two heads into zero-padded chunks
                            for hh in range(2):
                                h = 2 * dk + hh
                                po = hh * HD
                                ksl = dst[po:po + HD, h, si * SQ_T:(si + 1) * SQ_T]
                                if with_qk_bias:
                                    nc.scalar.activation(
                                        ksl, pp[po:po + HD, :], Identity,
                                        bias=bk_sb[po:po + HD, dk:dk + 1])
                                else:
                                    nc.vector.tensor_copy(ksl, pp[po:po + HD, :])

            # ---- v projection: natural [s, d] layout into v_aug ----
            for st in range(NSK):
                pv = psum.tile([P, D], f32, name="pv", tag="mm", bufs=4)
                n_acc = NE + (1 if with_v_bias else 0)
                for e in range(NE):
                    nc.tensor.matmul(
                        pv,
                        lhsT=xt_sb[e][:, st * P:(st + 1) * P],
                        rhs=wv_sb[:, e, :],
                        start=(e == 0),
                        stop=(e == n_acc - 1),
                    )
                if with_v_bias:
                    nc.tensor.matmul(pv, lhsT=ones_r, rhs=bv_row,
                                     start=False, stop=True)
                for h in range(H_PER):
                    nc.vector.tensor_copy(
                        v_aug[:, st, h, 0:HD], pv[:, h * HD:(h + 1) * HD])

            # ---- attention + out-projection, per sq block ----
            for si in range(NSQ):
                sq = slice(si * SQ_T, (si + 1) * SQ_T)
                ctxT = work.tile([P, ND, SQ_T], bf16, name="ctxT", tag="ctxT", bufs=2)
                for h in range(H_PER):
                    dk, po = h // 2, (h % 2) * HD
                    pc = psum.tile([HD + 1, SQ_T], f32, name="pc", tag="pc", bufs=2)
                    for sk in range(NSK):
                        ps = psum.tile([P, SQ_T], f32, name="ps", tag="mm", bufs=4)
                        nc.tensor.matmul(
                            ps,
                            lhsT=kT_sb[:, h, sk * SK_T:(sk + 1) * SK_T],
                            rhs=qT_sb[:, dk, sq],
                            start=True, stop=True,
                        )
                        ex = work.tile([P, SQ_T], bf16, name="ex", tag="ex", bufs=6)
                        nc.scalar.activation(ex, ps, Exp)
                        if with_mask:
                            mt = work.tile([P, SQ_T], bf16, name="mt", tag="mt", bufs=4)
                            nc.sync.dma_start(
                                mt, maskT_d[sk * SK_T:(sk + 1) * SK_T, sq])
                            nc.vector.tensor_mul(ex, ex, mt)
                        nc.tensor.matmul(
                            pc, lhsT=v_aug[:, sk, h, :], rhs=ex,
                            start=(sk == 0), stop=(sk == NSK - 1),
                        )
                    rec = work.tile([1, SQ_T], bf16, name="rec", tag="rec", bufs=2)
                    nc.vector.reciprocal(rec, pc[HD:HD + 1, :])
                    pb = psum.tile([HD, SQ_T], f32, name="pb", tag="pb", bufs=1)
                    nc.tensor.matmul(pb, lhsT=ones_sb, rhs=rec, start=True, stop=True)
                    cu = work.tile([HD, SQ_T], f32, name="cu", tag="cu", bufs=2)
                    nc.vector.tensor_copy(cu, pc[0:HD, :])
                    nc.vector.tensor_mul(ctxT[po:po + HD, dk, :], cu, pb)

                for do in range(NDO):
                    pu = psum.tile([P, SQ_T], f32, name="pu", tag="mm", bufs=4)
                    for dk in range(ND):
                        nc.tensor.matmul(
                            pu,
                            lhsT=wo_sb[:, dk, do * P:(do + 1) * P],
                            rhs=ctxT[:, dk, :],
                            start=(dk == 0), stop=(dk == ND - 1),
                        )
                    ot = work.tile([P, SQ_T], f32, name="ot", tag="ot", bufs=3)
                    nc.vector.tensor_copy(ot, pu)
                    nc.sync.dma_start(out_d[do * P:(do + 1) * P, sq], ot)

    nc.compile()
    return nc


_GRAPH_CACHE = {}


def kernel(x, mask, Wq, bq, Wk, bk, Wv, bv, Wo, bo):
    from concourse.bass_utils import run_bass_kernel_spmd

    x = np.asarray(x, dtype=np.float32)
    mask = np.asarray(mask)
    C = _quat_C()
    Mq = _expand(np.asarray(Wq), C) * SCALE
    Mk = _expand(np.asarray(Wk), C)
    Mv = _expand(np.asarray(Wv), C)
    Mo = _expand(np.asarray(Wo), C)
    bq_f = np.asarray(bq, np.float32).reshape(-1) * SCALE
    bk_f = np.asarray(bk, np.float32).reshape(-1)
    bv_f = np.asarray(bv, np.float32).reshape(-1)
    bo_f = np.asarray(bo, np.float32).reshape(-1)

    with_qk_bias = bool(np.any(bq_f) or np.any(bk_f))
    with_v_bias = bool(np.any(bv_f))
    with_mask = bool(np.any(np.asarray(mask) == 0))

    key = (with_qk_bias, with_v_bias, with_mask)
    if key not in _GRAPH_CACHE:
        _GRAPH_CACHE[key] = _build_graph(*key)
    nc = _GRAPH_CACHE[key]

    if with_mask:
        maskT = np.ascontiguousarray(
            np.broadcast_to(mask, (1, 1, S, S))[0, 0].T.astype(ml_dtypes.bfloat16))

    in_maps = []
    for core in range(8):
        b, hg = core // 4, core % 4
        cs = slice(hg * D, (hg + 1) * D)
        m = {
            "xt": np.ascontiguousarray(x[b].T.astype(ml_dtypes.bfloat16)),
            "wq": np.ascontiguousarray(Mq[:, cs].astype(ml_dtypes.bfloat16)),
            "wk": np.ascontiguousarray(Mk[:, cs].astype(ml_dtypes.bfloat16)),
            "wv": np.ascontiguousarray(Mv[:, cs].astype(ml_dtypes.bfloat16)),
            "wo": np.ascontiguousarray(Mo[cs, :].astype(ml_dtypes.bfloat16)),
        }
        if with_qk_bias:
            m["bq"] = np.ascontiguousarray(bq_f[cs])
            m["bk"] = np.ascontiguousarray(bk_f[cs])
        if with_v_bias:
            m["bv"] = np.ascontiguousarray(bv_f[cs].astype(ml_dtypes.bfloat16))
        if with_mask:
            m["maskT"] = maskT
        in_maps.append(m)

    res = run_bass_kernel_spmd(nc, in_maps, core_ids=list(range(8))).results

    out = np.zeros((B, S, E), dtype=np.float32)
    for core in range(8):
        b = core // 4
        out[b] += res[core]["out"].T
    out += bo_f
    return out


# revision 14
# speedup vs baseline: 1.6087x; 1.1560x over previous
"""Trainium2 Bass kernel for quaternion-algebra multi-head attention.

Math: algebra_linear(x, W, b) == x_flat @ M + b_flat where M[(n,j),(o,k)] =
sum_i C[i,j,k] W[o,n,i].  So the whole module is standard MHA with dense
1024x1024 projection matrices expanded on the host from the small algebra
weights.  Sharding: 8 cores = 2 batches x 4 head-groups (4 heads each).

Per-core device work (core c = (b, hg)):
  qT/kT [256, 2048] = Mq_shard^T-contraction against x^T (d on partitions)
  v     [2048, 256] natural layout, augmented with a ones column per head
  S^T[sk, sq] = kT.T @ qT per head (scale folded into Mq on host)
  expS = exp(S^T)  (no max subtraction: scores are bounded ~|4|, mask all-ones)
  ctx^T[65, sq] = [v | 1].T @ expS   -> rows 0:64 context, row 64 = softmax denom
  ctx_norm = ctx * (1/denom broadcast via K=1 matmul)
  out^T[1024, 2048] partial = Mo_shard^T-contraction against ctx_norm
Host gathers: out[b] = sum_hg out_hg^T.T (+ bo).
"""

import numpy as np
import ml_dtypes

B, S, E = 2, 2048, 1024
NB = 256          # algebra blocks
MD = 4            # quaternion dim
H = 16            # total heads
HD = 64           # head dim
H_PER = 4         # heads per core
D = 256           # head dims per core (H_PER * HD)
P = 128
NE = E // P       # 8 e-chunks
ND = D // P       # 2 d-chunks per core
SQ_T = 512
NSQ = S // SQ_T   # 4
SK_T = 128
NSK = S // SK_T   # 16
NDO = E // P      # 8 out-dim chunks
SCALE = 1.0 / np.sqrt(HD)

_QUAT_TABLE = [
    (0, 0, 0, 1.0), (0, 1, 1, 1.0), (0, 2, 2, 1.0), (0, 3, 3, 1.0),
    (1, 0, 1, 1.0), (2, 0, 2, 1.0), (3, 0, 3, 1.0),
    (1, 1, 0, -1.0), (2, 2, 0, -1.0), (3, 3, 0, -1.0),
    (1, 2, 3, 1.0), (2, 1, 3, -1.0),
    (2, 3, 1, 1.0), (3, 2, 1, -1.0),
    (3, 1, 2, 1.0), (1, 3, 2, -1.0),
]


def _quat_C():
    C = np.zeros((4, 4, 4), dtype=np.float32)
    for i, j, k, s in _QUAT_TABLE:
        C[i, j, k] = s
    return C


def _expand(W, C):
    # W [NB, NB, 4] -> dense [E, E]:  y_flat = x_flat @ M
    Wm = np.einsum('oni,ijk->onjk', W.astype(np.float32), C)
    return np.ascontiguousarray(Wm.transpose(1, 2, 0, 3).reshape(E, E))


def _build_graph(with_qk_bias, with_v_bias, with_mask):
    import concourse.bacc as bacc
    import concourse.tile as tile
    import concourse.mybir as mybir

    f32 = mybir.dt.float32
    bf16 = mybir.dt.bfloat16
    Exp = mybir.ActivationFunctionType.Exp
    Identity = mybir.ActivationFunctionType.Identity

    nc = bacc.Bacc("TRN2", target_bir_lowering=False, debug=False, num_devices=8)

    xt_d = nc.dram_tensor("xt", [E, S], bf16, kind="ExternalInput").ap()
    wq_d = nc.dram_tensor("wq", [E, D], bf16, kind="ExternalInput").ap()
    wk_d = nc.dram_tensor("wk", [E, D], bf16, kind="ExternalInput").ap()
    wv_d = nc.dram_tensor("wv", [E, D], bf16, kind="ExternalInput").ap()
    wo_d = nc.dram_tensor("wo", [D, E], bf16, kind="ExternalInput").ap()
    out_d = nc.dram_tensor("out", [E, S], f32, kind="ExternalOutput").ap()
    if with_qk_bias:
        bq_d = nc.dram_tensor("bq", [D], f32, kind="ExternalInput").ap()
        bk_d = nc.dram_tensor("bk", [D], f32, kind="ExternalInput").ap()
    if with_v_bias:
        bv_d = nc.dram_tensor("bv", [D], bf16, kind="ExternalInput").ap()
    if with_mask:
        maskT_d = nc.dram_tensor("maskT", [S, S], bf16, kind="ExternalInput").ap()

    with tile.TileContext(nc) as tc:
        import contextlib
        with nc.allow_low_precision(reason="float32r rounding of matmul operands"), \
                contextlib.ExitStack() as ctx:
            sing = ctx.enter_context(tc.tile_pool(name="sing", bufs=1))
            psum = ctx.enter_context(tc.tile_pool(name="psum", bufs=1, space="PSUM"))
            work = ctx.enter_context(tc.tile_pool(name="work", bufs=1))

            # ---- persistent SBUF tiles ----
            xt_sb = [
                sing.tile([P, S], bf16, name=f"xt{e}", tag=f"xt{e}")
                for e in range(NE)
            ]
            wq_sb = sing.tile([P, NE, D], bf16, name="wq_sb", tag="wq_sb")
            wk_sb = sing.tile([P, NE, D], bf16, name="wk_sb", tag="wk_sb")
            wv_sb = sing.tile([P, NE, D], bf16, name="wv_sb", tag="wv_sb")
            wo_sb = sing.tile([P, ND, E], bf16, name="wo_sb", tag="wo_sb")
            qT_sb = sing.tile([P, ND, S], bf16, name="qT_sb", tag="qT_sb")
            kT_sb = sing.tile([P, H_PER, S], bf16, name="kT_sb", tag="kT_sb")
            nc.vector.memset(kT_sb, 0.0)
            v_aug = sing.tile([P, NSK, H_PER, P], bf16, name="v_aug", tag="v_aug")
            ones_sb = sing.tile([1, HD], bf16, name="ones_sb", tag="ones_sb")

            # ---- input DMAs ----
            nc.sync.dma_start(wq_sb, wq_d.rearrange("(ko p) d -> p ko d", p=P))
            nc.sync.dma_start(wk_sb, wk_d.rearrange("(ko p) d -> p ko d", p=P))
            nc.sync.dma_start(wv_sb, wv_d.rearrange("(ko p) d -> p ko d", p=P))
            nc.sync.dma_start(wo_sb, wo_d.rearrange("(dk p) o -> p dk o", p=P))
            for e in range(NE):
                nc.sync.dma_start(xt_sb[e], xt_d[e * P:(e + 1) * P, :])

            import ml_dtypes
            ones_init = nc.inline_tensor(
                np.ones((P, NSK, H_PER, P), ml_dtypes.bfloat16), name="ones_init").ap()
            ones_row = nc.inline_tensor(
                np.ones((1, P), ml_dtypes.bfloat16), name="ones_row").ap()
            nc.sync.dma_start(v_aug, ones_init)
            nc.sync.dma_start(ones_sb, ones_row[:, :HD])

            if with_qk_bias:
                bq_sb = sing.tile([P, ND], f32, name="bq_sb", tag="bq_sb")
                bk_sb = sing.tile([P, ND], f32, name="bk_sb", tag="bk_sb")
                nc.sync.dma_start(bq_sb, bq_d.rearrange("(dk p) -> p dk", p=P))
                nc.sync.dma_start(bk_sb, bk_d.rearrange("(dk p) -> p dk", p=P))
            if with_v_bias:
                bv_row = sing.tile([1, D], bf16, name="bv_row", tag="bv_row")
                ones_r = sing.tile([1, P], bf16, name="ones_r", tag="ones_r")
                nc.sync.dma_start(bv_row, bv_d[None, :])
                nc.sync.dma_start(ones_r, ones_row)

            # ---- q/k projections: qT[d, s] accumulated over e-chunks ----
            for wsb, dst, bias_sb in (
                (wq_sb, qT_sb, "bq_sb"),
                (wk_sb, kT_sb, "bk_sb"),
            ):
                for dk in range(ND):
                    for si in range(NSQ):
                        pp = psum.tile([P, SQ_T], f32, name="pp", tag="mm", bufs=4)
                        for e in range(NE):
                            nc.tensor.matmul(
                                pp,
                                lhsT=wsb[:, e, dk * P:(dk + 1) * P],
                                rhs=xt_sb[e][:, si * SQ_T:(si + 1) * SQ_T],
                                start=(e == 0),
                                stop=(e == NE - 1),
                            )
                        if dst is qT_sb:
                            dslice = dst[:, dk, si * SQ_T:(si + 1) * SQ_T]
                            if with_qk_bias:
                                nc.scalar.activation(
                                    dslice, pp, Identity, bias=bq_sb[:, dk:dk + 1])
                            else:
                                nc.vector.tensor_copy(dslice, pp)
                        else:
                            # kT: split the two heads into zero-padded chunks
                            for hh in range(2):
                                h = 2 * dk + hh
                                po = hh * HD
                                ksl = dst[po:po + HD, h, si * SQ_T:(si + 1) * SQ_T]
                                if with_qk_bias:
                                    nc.scalar.activation(
                                        ksl, pp[po:po + HD, :], Identity,
                                        bias=bk_sb[po:po + HD, dk:dk + 1])
                                else:
                                    nc.vector.tensor_copy(ksl, pp[po:po + HD, :])

            # ---- v projection: natural [s, d] layout into v_aug ----
            for st in range(NSK):
                pv = psum.tile([P, D], f32, name="pv", tag="mm", bufs=4)
                n_acc = NE + (1 if with_v_bias else 0)
                for e in range(NE):
                    nc.tensor.matmul(
                        pv,
                        lhsT=xt_sb[e][:, st * P:(st + 1) * P],
                        rhs=wv_sb[:, e, :],
                        start=(e == 0),
                        stop=(e == n_acc - 1),
                    )
                if with_v_bias:
                    nc.tensor.matmul(pv, lhsT=ones_r, rhs=bv_row,
                                     start=False, stop=True)
                for h in range(H_PER):
                    nc.vector.tensor_copy(
                        v_aug[:, st, h, HD:2 * HD], pv[:, h * HD:(h + 1) * HD])

            # ---- attention + out-projection, per sq block ----
            for si in range(NSQ):
                sq = slice(si * SQ_T, (si + 1) * SQ_T)
                ctxT = work.tile([P, ND, SQ_T], bf16, name="ctxT", tag="ctxT", bufs=2)
                for h in range(H_PER):
                    dk, po = h // 2, (h % 2) * HD
                    pc = psum.tile([P, SQ_T], f32, name="pc", tag="pc", bufs=2)
                    for sk in range(NSK):
                        ps = psum.tile([P, SQ_T], f32, name="ps", tag="mm", bufs=4)
                        nc.tensor.matmul(
                            ps,
                            lhsT=kT_sb[:, h, sk * SK_T:(sk + 1) * SK_T],
                            rhs=qT_sb[:, dk, sq],
                            start=True, stop=True,
                        )
                        ex = work.tile([P, SQ_T], bf16, name="ex", tag="ex", bufs=6)
                        nc.scalar.activation(ex, ps, Exp)
                        if with_mask:
                            mt = work.tile([P, SQ_T], bf16, name="mt", tag="mt", bufs=4)
                            nc.sync.dma_start(
                                mt, maskT_d[sk * SK_T:(sk + 1) * SK_T, sq])
                            nc.vector.tensor_mul(ex, ex, mt)
                        nc.tensor.matmul(
                            pc, lhsT=v_aug[:, sk, h, :], rhs=ex,
                            start=(sk == 0), stop=(sk == NSK - 1),
                        )
                    recf = work.tile([1, SQ_T], f32, name="recf", tag="recf", bufs=2)
                    nc.vector.reciprocal_approx_fast(recf, pc[0:1, :])
                    rec = work.tile([1, SQ_T], bf16, name="rec", tag="rec", bufs=2)
                    nc.vector.tensor_copy(rec, recf)
                    pb = psum.tile([HD, SQ_T], f32, name="pb", tag="pb", bufs=1)
                    nc.tensor.matmul(pb, lhsT=ones_sb, rhs=rec, start=True, stop=True)
                    cu = work.tile([HD, SQ_T], f32, name="cu", tag="cu", bufs=2)
                    nc.vector.tensor_copy(cu, pc[HD:2 * HD, :])
                    nc.vector.tensor_mul(ctxT[po:po + HD, dk, :], cu, pb)

                for do in range(NDO):
                    pu = psum.tile([P, SQ_T], f32, name="pu", tag="mm", bufs=4)
                    for dk in range(ND):
                        nc.tensor.matmul(
                            pu,
                            lhsT=wo_sb[:, dk, do * P:(do + 1) * P],
                            rhs=ctxT[:, dk, :],
                            start=(dk == 0), stop=(dk == ND - 1),
                        )
                    ot = work.tile([P, SQ_T], f32, name="ot", tag="ot", bufs=3)
                    nc.vector.tensor_copy(ot, pu)
                    nc.sync.dma_start(out_d[do * P:(do + 1) * P, sq], ot)

    nc.compile()
    return nc


_GRAPH_CACHE = {}


def kernel(x, mask, Wq, bq, Wk, bk, Wv, bv, Wo, bo):
    from concourse.bass_utils import run_bass_kernel_spmd

    x = np.asarray(x, dtype=np.float32)
    mask = np.asarray(mask)
    C = _quat_C()
    Mq = _expand(np.asarray(Wq), C) * SCALE
    Mk = _expand(np.asarray(Wk), C)
    Mv = _expand(np.asarray(Wv), C)
    Mo = _expand(np.asarray(Wo), C)
    bq_f = np.asarray(bq, np.float32).reshape(-1) * SCALE
    bk_f = np.asarray(bk, np.float32).reshape(-1)
    bv_f = np.asarray(bv, np.float32).reshape(-1)
    bo_f = np.asarray(bo, np.float32).reshape(-1)

    with_qk_bias = bool(np.any(bq_f) or np.any(bk_f))
    with_v_bias = bool(np.any(bv_f))
    with_mask = bool(np.any(np.asarray(mask) == 0))

    key = (with_qk_bias, with_v_bias, with_mask)
    if key not in _GRAPH_CACHE:
        _GRAPH_CACHE[key] = _build_graph(*key)
    nc = _GRAPH_CACHE[key]

    if with_mask:
        maskT = np.ascontiguousarray(
            np.broadcast_to(mask, (1, 1, S, S))[0, 0].T.astype(ml_dtypes.bfloat16))

    in_maps = []
    for core in range(8):
        b, hg = core // 4, core % 4
        cs = slice(hg * D, (hg + 1) * D)
        m = {
            "xt": np.ascontiguousarray(x[b].T.astype(ml_dtypes.bfloat16)),
            "wq": np.ascontiguousarray(Mq[:, cs].astype(ml_dtypes.bfloat16)),
            "wk": np.ascontiguousarray(Mk[:, cs].astype(ml_dtypes.bfloat16)),
            "wv": np.ascontiguousarray(Mv[:, cs].astype(ml_dtypes.bfloat16)),
            "wo": np.ascontiguousarray(Mo[cs, :].astype(ml_dtypes.bfloat16)),
        }
        if with_qk_bias:
            m["bq"] = np.ascontiguousarray(bq_f[cs])
            m["bk"] = np.ascontiguousarray(bk_f[cs])
        if with_v_bias:
            m["bv"] = np.ascontiguousarray(bv_f[cs].astype(ml_dtypes.bfloat16))
        if with_mask:
            m["maskT"] = maskT
        in_maps.append(m)

    res = run_bass_kernel_spmd(nc, in_maps, core_ids=list(range(8))).results

    out = np.zeros((B, S, E), dtype=np.float32)
    for core in range(8):
        b = core // 4
        out[b] += res[core]["out"].T
    out += bo_f
    return out


# revision 15
# speedup vs baseline: 1.7174x; 1.0676x over previous
"""Trainium2 Bass kernel for quaternion-algebra multi-head attention.

Math: algebra_linear(x, W, b) == x_flat @ M + b_flat where M[(n,j),(o,k)] =
sum_i C[i,j,k] W[o,n,i].  So the whole module is standard MHA with dense
1024x1024 projection matrices expanded on the host from the small algebra
weights.  Sharding: 8 cores = 2 batches x 4 head-groups (4 heads each).

Per-core device work (core c = (b, hg)):
  qT/kT [256, 2048] = Mq_shard^T-contraction against x^T (d on partitions)
  v     [2048, 256] natural layout, augmented with a ones column per head
  S^T[sk, sq] = kT.T @ qT per head (scale folded into Mq on host)
  expS = exp(S^T)  (no max subtraction: scores are bounded ~|4|, mask all-ones)
  ctx^T[65, sq] = [v | 1].T @ expS   -> rows 0:64 context, row 64 = softmax denom
  ctx_norm = ctx * (1/denom broadcast via K=1 matmul)
  out^T[1024, 2048] partial = Mo_shard^T-contraction against ctx_norm
Host gathers: out[b] = sum_hg out_hg^T.T (+ bo).
"""

import numpy as np
import ml_dtypes

B, S, E = 2, 2048, 1024
NB = 256          # algebra blocks
MD = 4            # quaternion dim
H = 16            # total heads
HD = 64           # head dim
H_PER = 4         # heads per core
D = 256           # head dims per core (H_PER * HD)
P = 128
NE = E // P       # 8 e-chunks
ND = D // P       # 2 d-chunks per core
SQ_T = 512
NSQ = S // SQ_T   # 4
SK_T = 128
NSK = S // SK_T   # 16
NDO = E // P      # 8 out-dim chunks
SCALE = 1.0 / np.sqrt(HD)

_QUAT_TABLE = [
    (0, 0, 0, 1.0), (0, 1, 1, 1.0), (0, 2, 2, 1.0), (0, 3, 3, 1.0),
    (1, 0, 1, 1.0), (2, 0, 2, 1.0), (3, 0, 3, 1.0),
    (1, 1, 0, -1.0), (2, 2, 0, -1.0), (3, 3, 0, -1.0),
    (1, 2, 3, 1.0), (2, 1, 3, -1.0),
    (2, 3, 1, 1.0), (3, 2, 1, -1.0),
    (3, 1, 2, 1.0), (1, 3, 2, -1.0),
]


def _quat_C():
    C = np.zeros((4, 4, 4), dtype=np.float32)
    for i, j, k, s in _QUAT_TABLE:
        C[i, j, k] = s
    return C


def _expand(W, C):
    # W [NB, NB, 4] -> dense [E, E]:  y_flat = x_flat @ M
    Wm = np.einsum('oni,ijk->onjk', W.astype(np.float32), C)
    return np.ascontiguousarray(Wm.transpose(1, 2, 0, 3).reshape(E, E))


def _build_graph(with_qk_bias, with_v_bias, with_mask):
    import concourse.bacc as bacc
    import concourse.tile as tile
    import concourse.mybir as mybir

    f32 = mybir.dt.float32
    bf16 = mybir.dt.bfloat16
    Exp = mybir.ActivationFunctionType.Exp
    Identity = mybir.ActivationFunctionType.Identity

    nc = bacc.Bacc("TRN2", target_bir_lowering=False, debug=False, num_devices=8)

    xt_d = nc.dram_tensor("xt", [E, S], bf16, kind="ExternalInput").ap()
    wq_d = nc.dram_tensor("wq", [E, D], bf16, kind="ExternalInput").ap()
    wk_d = nc.dram_tensor("wk", [E, D], bf16, kind="ExternalInput").ap()
    wv_d = nc.dram_tensor("wv", [E, D], bf16, kind="ExternalInput").ap()
    wo_d = nc.dram_tensor("wo", [D, E], bf16, kind="ExternalInput").ap()
    out_d = nc.dram_tensor("out", [E, S], f32, kind="ExternalOutput").ap()
    if with_qk_bias:
        bq_d = nc.dram_tensor("bq", [D], f32, kind="ExternalInput").ap()
        bk_d = nc.dram_tensor("bk", [D], f32, kind="ExternalInput").ap()
    if with_v_bias:
        bv_d = nc.dram_tensor("bv", [D], bf16, kind="ExternalInput").ap()
    if with_mask:
        maskT_d = nc.dram_tensor("maskT", [S, S], bf16, kind="ExternalInput").ap()

    with tile.TileContext(nc) as tc:
        import contextlib
        with nc.allow_low_precision(reason="float32r rounding of matmul operands"), \
                contextlib.ExitStack() as ctx:
            sing = ctx.enter_context(tc.tile_pool(name="sing", bufs=1))
            psum = ctx.enter_context(tc.tile_pool(name="psum", bufs=1, space="PSUM"))
            work = ctx.enter_context(tc.tile_pool(name="work", bufs=1))

            # ---- persistent SBUF tiles ----
            xt_sb = [
                sing.tile([P, S], bf16, name=f"xt{e}", tag=f"xt{e}")
                for e in range(NE)
            ]
            wq_sb = sing.tile([P, NE, D], bf16, name="wq_sb", tag="wq_sb")
            wk_sb = sing.tile([P, NE, D], bf16, name="wk_sb", tag="wk_sb")
            wv_sb = sing.tile([P, NE, D], bf16, name="wv_sb", tag="wv_sb")
            wo_sb = sing.tile([P, ND, E], bf16, name="wo_sb", tag="wo_sb")
            qT_sb = sing.tile([P, ND, S], bf16, name="qT_sb", tag="qT_sb")
            kT_sb = sing.tile([P, ND, S], bf16, name="kT_sb", tag="kT_sb")
            v_aug = sing.tile([P, NSK, H_PER, P], bf16, name="v_aug", tag="v_aug")
            ones_sb = sing.tile([1, HD], bf16, name="ones_sb", tag="ones_sb")

            # ---- input DMAs ----
            nc.sync.dma_start(wq_sb, wq_d.rearrange("(ko p) d -> p ko d", p=P))
            nc.sync.dma_start(wk_sb, wk_d.rearrange("(ko p) d -> p ko d", p=P))
            nc.sync.dma_start(wv_sb, wv_d.rearrange("(ko p) d -> p ko d", p=P))
            nc.sync.dma_start(wo_sb, wo_d.rearrange("(dk p) o -> p dk o", p=P))
            for e in range(NE):
                nc.sync.dma_start(xt_sb[e], xt_d[e * P:(e + 1) * P, :])

            import ml_dtypes
            ones_init = nc.inline_tensor(
                np.ones((P, NSK, H_PER, P), ml_dtypes.bfloat16), name="ones_init").ap()
            ones_row = nc.inline_tensor(
                np.ones((1, P), ml_dtypes.bfloat16), name="ones_row").ap()
            nc.sync.dma_start(v_aug, ones_init)
            nc.sync.dma_start(ones_sb, ones_row[:, :HD])

            if with_qk_bias:
                bq_sb = sing.tile([P, ND], f32, name="bq_sb", tag="bq_sb")
                bk_sb = sing.tile([P, ND], f32, name="bk_sb", tag="bk_sb")
                nc.sync.dma_start(bq_sb, bq_d.rearrange("(dk p) -> p dk", p=P))
                nc.sync.dma_start(bk_sb, bk_d.rearrange("(dk p) -> p dk", p=P))
            if with_v_bias:
                bv_row = sing.tile([1, D], bf16, name="bv_row", tag="bv_row")
                ones_r = sing.tile([1, P], bf16, name="ones_r", tag="ones_r")
                nc.sync.dma_start(bv_row, bv_d[None, :])
                nc.sync.dma_start(ones_r, ones_row)

            # ---- q/k projections: qT[d, s] accumulated over e-chunks ----
            for wsb, dst, bias_sb in (
                (wq_sb, qT_sb, "bq_sb"),
                (wk_sb, kT_sb, "bk_sb"),
            ):
                for dk in range(ND):
                    for si in range(NSQ):
                        pp = psum.tile([P, SQ_T], f32, name="pp", tag="mm", bufs=2)
                        for e in range(NE):
                            nc.tensor.matmul(
                                pp,
                                lhsT=wsb[:, e, dk * P:(dk + 1) * P],
                                rhs=xt_sb[e][:, si * SQ_T:(si + 1) * SQ_T],
                                start=(e == 0),
                                stop=(e == NE - 1),
                            )
                        dslice = dst[:, dk, si * SQ_T:(si + 1) * SQ_T]
                        if with_qk_bias:
                            bb = bq_sb if bias_sb == "bq_sb" else bk_sb
                            nc.scalar.activation(
                                dslice, pp, Identity, bias=bb[:, dk:dk + 1])
                        else:
                            nc.vector.tensor_copy(dslice, pp)

            # ---- v projection: natural [s, d] layout into v_aug ----
            for st in range(NSK):
                pv = psum.tile([P, D], f32, name="pv", tag="mm", bufs=2)
                n_acc = NE + (1 if with_v_bias else 0)
                for e in range(NE):
                    nc.tensor.matmul(
                        pv,
                        lhsT=xt_sb[e][:, st * P:(st + 1) * P],
                        rhs=wv_sb[:, e, :],
                        start=(e == 0),
                        stop=(e == n_acc - 1),
                    )
                if with_v_bias:
                    nc.tensor.matmul(pv, lhsT=ones_r, rhs=bv_row,
                                     start=False, stop=True)
                for h in range(H_PER):
                    nc.vector.tensor_copy(
                        v_aug[:, st, h, HD:2 * HD], pv[:, h * HD:(h + 1) * HD])

            # ---- attention + out-projection, per sq block ----
            # heads processed in pairs: two K=64 score matmuls run
            # concurrently in the top/bottom halves of the PE array
            for si in range(NSQ):
                sq = slice(si * SQ_T, (si + 1) * SQ_T)
                ctxT = work.tile([P, ND, SQ_T], bf16, name="ctxT", tag="ctxT", bufs=2)
                for dk in range(ND):
                    pcs = [
                        psum.tile([P, SQ_T], f32, name=f"pc{j}", tag="pc", bufs=2)
                        for j in range(2)
                    ]
                    for sk in range(NSK):
                        ps = psum.tile([P, 2, SQ_T], f32, name="ps", tag="sc", bufs=2)
                        for j in range(2):
                            po = j * HD
                            nc.tensor.matmul(
                                ps[:, j, :],
                                lhsT=kT_sb[po:po + HD, dk, sk * SK_T:(sk + 1) * SK_T],
                                rhs=qT_sb[po:po + HD, dk, sq],
                                start=True, stop=True,
                                tile_position=(po, 0),
                            )
                        ex = work.tile([P, 2, SQ_T], bf16, name="ex", tag="ex", bufs=4)
                        nc.scalar.activation(ex, ps, Exp)
                        if with_mask:
                            mt = work.tile([P, SQ_T], bf16, name="mt", tag="mt", bufs=4)
                            nc.sync.dma_start(
                                mt, maskT_d[sk * SK_T:(sk + 1) * SK_T, sq])
                            nc.vector.tensor_mul(
                                ex, ex, mt[:, None, :].to_broadcast([P, 2, SQ_T]))
                        for j in range(2):
                            nc.tensor.matmul(
                                pcs[j], lhsT=v_aug[:, sk, 2 * dk + j, :],
                                rhs=ex[:, j, :],
                                start=(sk == 0), stop=(sk == NSK - 1),
                            )
                    for j in range(2):
                        pc, po = pcs[j], j * HD
                        recf = work.tile([1, SQ_T], f32, name="recf", tag="recf", bufs=2)
                        nc.vector.reciprocal_approx_fast(recf, pc[0:1, :])
                        rec = work.tile([1, SQ_T], bf16, name="rec", tag="rec", bufs=2)
                        nc.vector.tensor_copy(rec, recf)
                        pb = psum.tile([HD, SQ_T], f32, name="pb", tag="mm", bufs=2)
                        nc.tensor.matmul(pb, lhsT=ones_sb, rhs=rec, start=True, stop=True)
                        cu = work.tile([HD, SQ_T], f32, name="cu", tag="cu", bufs=2)
                        nc.vector.tensor_copy(cu, pc[HD:2 * HD, :])
                        nc.vector.tensor_mul(ctxT[po:po + HD, dk, :], cu, pb)

                for do in range(NDO):
                    pu = psum.tile([P, SQ_T], f32, name="pu", tag="mm", bufs=2)
                    for dk in range(ND):
                        nc.tensor.matmul(
                            pu,
                            lhsT=wo_sb[:, dk, do * P:(do + 1) * P],
                            rhs=ctxT[:, dk, :],
                            start=(dk == 0), stop=(dk == ND - 1),
                        )
                    ot = work.tile([P, SQ_T], f32, name="ot", tag="ot", bufs=3)
                    nc.vector.tensor_copy(ot, pu)
                    nc.sync.dma_start(out_d[do * P:(do + 1) * P, sq], ot)

    nc.compile()
    return nc


_GRAPH_CACHE = {}


def kernel(x, mask, Wq, bq, Wk, bk, Wv, bv, Wo, bo):
    from concourse.bass_utils import run_bass_kernel_spmd

    x = np.asarray(x, dtype=np.float32)
    mask = np.asarray(mask)
    C = _quat_C()
    Mq = _expand(np.asarray(Wq), C) * SCALE
    Mk = _expand(np.asarray(Wk), C)
    Mv = _expand(np.asarray(Wv), C)
    Mo = _expand(np.asarray(Wo), C)
    bq_f = np.asarray(bq, np.float32).reshape(-1) * SCALE
    bk_f = np.asarray(bk, np.float32).reshape(-1)
    bv_f = np.asarray(bv, np.float32).reshape(-1)
    bo_f = np.asarray(bo, np.float32).reshape(-1)

    with_qk_bias = bool(np.any(bq_f) or np.any(bk_f))
    with_v_bias = bool(np.any(bv_f))
    with_mask = bool(np.any(np.asarray(mask) == 0))

    key = (with_qk_bias, with_v_bias, with_mask)
    if key not in _GRAPH_CACHE:
        _GRAPH_CACHE[key] = _build_graph(*key)
    nc = _GRAPH_CACHE[key]

    if with_mask:
        maskT = np.ascontiguousarray(
            np.broadcast_to(mask, (1, 1, S, S))[0, 0].T.astype(ml_dtypes.bfloat16))

    in_maps = []
    for core in range(8):
        b, hg = core // 4, core % 4
        cs = slice(hg * D, (hg + 1) * D)
        m = {
            "xt": np.ascontiguousarray(x[b].T.astype(ml_dtypes.bfloat16)),
            "wq": np.ascontiguousarray(Mq[:, cs].astype(ml_dtypes.bfloat16)),
            "wk": np.ascontiguousarray(Mk[:, cs].astype(ml_dtypes.bfloat16)),
            "wv": np.ascontiguousarray(Mv[:, cs].astype(ml_dtypes.bfloat16)),
            "wo": np.ascontiguousarray(Mo[cs, :].astype(ml_dtypes.bfloat16)),
        }
        if with_qk_bias:
            m["bq"] = np.ascontiguousarray(bq_f[cs])
            m["bk"] = np.ascontiguousarray(bk_f[cs])
        if with_v_bias:
            m["bv"] = np.ascontiguousarray(bv_f[cs].astype(ml_dtypes.bfloat16))
        if with_mask:
            m["maskT"] = maskT
        in_maps.append(m)

    res = run_bass_kernel_spmd(nc, in_maps, core_ids=list(range(8))).results

    out = np.zeros((B, S, E), dtype=np.float32)
    for core in range(8):
        b = core // 4
        out[b] += res[core]["out"].T
    out += bo_f
    return out
